# revision 3
# baseline (speedup 1.0000x reference)
"""Trainium2 Bass kernel for nn_DualModel (B=2,S=2048,V=32000,D=256).

Sharding: 8 cores = 2 batch groups x 4 vocab shards. Core c handles
batch c//4 and vocab columns [8000*(c%4), 8000*(c%4+1)). Each core
runs the full transformer for its batch (embed + 2 attention layers)
and the logits GEMM for its vocab shard. No inter-core communication.

Device layout: activations kept transposed ("X^T" = [D, S]) in SBUF so
every matmul consumes them directly (lhsT for scores/values/logits, rhs
for Q projection). Attention probabilities are computed transposed
(A^T[t, q] = exp(16 * K_t . Q_q)), softmax normalization is folded in
after the A^T @ V matmul (it is linear per query column). The FFN
residual (X + X @ WF^T) is folded into the unembedding on the host:
logits = X2 @ (U + U @ WF)^T exactly.

Matmuls run in float32r (full PE rate); operands are rounded to f32r by
their producers (DMA from f32r DRAM, DVE/ACT writes to f32r tiles).
"""

import numpy as np

import concourse.bacc as bacc
import concourse.bass as bass
import concourse.mybir as mybir
import concourse.tile as tile
from concourse.bass_utils import run_bass_kernel_spmd
from concourse.masks import make_identity

P = 128
B, S, V, D = 2, 2048, 32000, 256
NCORES = 8
CPG = 4               # cores per batch group (vocab shards)
VSH = V // CPG        # 8000 vocab columns per core
KO = D // P           # 2 contraction chunks of 128
TB = S // P           # 16 token tiles / key chunks
QC = 512              # query-chunk width
NQ = S // QC          # 4 query chunks
VC = 500              # logits vocab chunk (PSUM bank limit 512 fp32)
NVC = VSH // VC       # 16
NEG = -1e30

F32 = mybir.dt.float32
F32R = mybir.dt.float32r
I32 = mybir.dt.int32

_CACHE = {}


def _classify(mask):
    """Per-128x128 block classification of mask[b][query, key], merged
    across batches into one SPMD-shared structure.

    Returns (structure, per_batch_tiles):
      structure: dict with
        strips[n]  -> ordered list of key-chunks i computed for q-chunk n
        blocks[(i, j)] -> 'plain' | 'zero' | ('add', tile_idx)
        n_mix      -> number of additive tiles
      per_batch_tiles: [B] list of np arrays [n_mix, P, P] fp32
    """
    stat = np.empty((B, TB, TB), dtype=np.int8)  # [b, key i, query j]
    mix = {}
    for b in range(B):
        mb = np.asarray(mask[b], dtype=bool)
        for j in range(TB):
            for i in range(TB):
                blk = mb[j * P:(j + 1) * P, i * P:(i + 1) * P]
                if blk.all():
                    stat[b, i, j] = 2
                elif not blk.any():
                    stat[b, i, j] = 0
                else:
                    stat[b, i, j] = 1
                    mix[(b, i, j)] = np.where(blk.T, 0.0, NEG).astype(np.float32)

    def tile_for(b, i, j):
        st = stat[b, i, j]
        if st == 2:
            return np.zeros((P, P), np.float32)
        if st == 0:
            return np.full((P, P), NEG, np.float32)
        return mix[(b, i, j)]

    blocks = {}
    dedupe = {}
    per_batch = [[] for _ in range(B)]
    for i in range(TB):
        for j in range(TB):
            sts = stat[:, i, j]
            if (sts == 2).all():
                blocks[(i, j)] = "plain"
            elif (sts == 0).all():
                blocks[(i, j)] = "zero"
            else:
                ts = [tile_for(b, i, j) for b in range(B)]
                key = tuple(t.tobytes() for t in ts)
                if key not in dedupe:
                    dedupe[key] = len(dedupe)
                    for b in range(B):
                        per_batch[b].append(ts[b])
                blocks[(i, j)] = ("add", dedupe[key])

    strips = []
    for n in range(NQ):
        js = range(4 * n, 4 * n + 4)
        strips.append(
            [i for i in range(TB) if any(blocks[(i, j)] != "zero" for j in js)]
        )
    n_mix = len(dedupe)
    structure = {
        "strips": tuple(tuple(s) for s in strips),
        "blocks": blocks,
        "n_mix": n_mix,
    }
    tiles = [
        np.stack(per_batch[b]) if n_mix else np.zeros((1, P, P), np.float32)
        for b in range(B)
    ]
    return structure, tiles


def _build(structure):
    strips = structure["strips"]
    blocks = structure["blocks"]
    n_mix = max(structure["n_mix"], 1)

    nc = bacc.Bacc("TRN2", target_bir_lowering=False, debug=False,
                   num_devices=NCORES)

    tok_d = nc.dram_tensor("tok", [S], I32, kind="ExternalInput")
    emb_d = nc.dram_tensor("emb", [V, D], F32, kind="ExternalInput")
    pos_d = nc.dram_tensor("pos", [S, D], F32, kind="ExternalInput")
    w_d = {
        n: nc.dram_tensor(n, [D, D], F32R, kind="ExternalInput")
        for n in ("wqk1", "wov1", "wqk2", "wov2")
    }
    ut_d = nc.dram_tensor("ut", [D, VSH], F32R, kind="ExternalInput")
    cm_d = nc.dram_tensor("cmadd", [n_mix, P, P], F32, kind="ExternalInput")
    out_d = nc.dram_tensor("out", [S, VSH], F32, kind="ExternalOutput")

    with tile.TileContext(nc) as tc:
        with (
            tc.tile_pool(name="cpool", bufs=1) as cpool,
            tc.tile_pool(name="xpool", bufs=1) as xpool,
            tc.tile_pool(name="upool", bufs=1) as upool,
            tc.tile_pool(name="qpool", bufs=2) as qpool,
            tc.tile_pool(name="epool", bufs=3) as epool,
            tc.tile_pool(name="apool", bufs=3) as apool,
            tc.tile_pool(name="npool", bufs=2) as npool,
            tc.tile_pool(name="opool", bufs=4) as opool,
            tc.tile_pool(name="psA", bufs=3, space="PSUM") as psA,
            tc.tile_pool(name="psS", bufs=2, space="PSUM") as psS,
            tc.tile_pool(name="psY", bufs=1, space="PSUM") as psY,
            tc.tile_pool(name="psR", bufs=1, space="PSUM") as psR,
        ):
            # ---- constants ----
            ident = cpool.tile([P, P], F32)
            make_identity(nc, ident[:])
            ones_f = cpool.tile([P, 1], F32)
            nc.vector.memset(ones_f[:], 1.0)
            ones_r = cpool.tile([P, 1], F32R)
            nc.vector.tensor_copy(ones_r[:], ones_f[:])
            toks = cpool.tile([P, TB], I32)
            nc.sync.dma_start(toks[:], tok_d.rearrange("(o p) -> p o", p=P))
            w = {}
            for nme in w_d:
                w[nme] = cpool.tile([P, KO, D], F32R, name=f"w_{nme}")
                nc.sync.dma_start(
                    w[nme][:], w_d[nme].rearrange("(ko p) n -> p ko n", p=P)
                )
            cm = cpool.tile([P, n_mix, P], F32)
            nc.sync.dma_start(cm[:], cm_d.rearrange("n p q -> p n q"))

            # unembedding, resident in SBUF (read once from HBM)
            ut = []
            for vci in range(NVC):
                t = upool.tile([P, KO, VC], F32R, name=f"ut{vci}", tag=f"ut{vci}")
                nc.sync.dma_start(
                    t[:],
                    ut_d.rearrange("(ko p) v -> p ko v", p=P)[
                        :, :, vci * VC:(vci + 1) * VC
                    ],
                )
                ut.append(t)

            # activation buffers, [P, KO, QC] per query chunk
            xa = [xpool.tile([P, KO, QC], F32R, name=f"xa{n}", tag=f"xa{n}") for n in range(NQ)]
            xb = [xpool.tile([P, KO, QC], F32R, name=f"xb{n}", tag=f"xb{n}") for n in range(NQ)]

            # ---- embedding: X0^T = (E[tok] + pos)^T ----
            for i in range(TB):
                g = epool.tile([P, D], F32, tag="g")
                nc.gpsimd.indirect_dma_start(
                    out=g[:],
                    out_offset=None,
                    in_=emb_d[:],
                    in_offset=bass.IndirectOffsetOnAxis(ap=toks[:, i:i + 1], axis=0),
                )
                pt = epool.tile([P, D], F32, tag="pt")
                nc.sync.dma_start(pt[:], pos_d[P * i:P * (i + 1), :])
                x0 = epool.tile([P, D], F32, tag="x0")
                nc.vector.tensor_add(x0[:], g[:], pt[:])
                for k in range(KO):
                    tps = psA.tile([P, P], F32, tag="mm")
                    nc.tensor.transpose(tps[:], x0[:, P * k:P * (k + 1)], ident[:])
                    nc.vector.tensor_copy(
                        xa[i // 4][:, k, P * (i % 4):P * (i % 4 + 1)], tps[:]
                    )

            # ---- attention layer ----
            def layer(cur, nxt, wqk, wov):
                # V = X @ WOV^T in [t, d] layout (lhsT for the A^T matmul)
                vt = []
                for i in range(TB):
                    ps = psA.tile([P, D], F32, tag="mm")
                    for k in range(KO):
                        nc.tensor.matmul(
                            ps[:],
                            cur[i // 4][:, k, P * (i % 4):P * (i % 4 + 1)],
                            wov[:, k, :],
                            start=(k == 0),
                            stop=(k == KO - 1),
                        )
                    t = xpool.tile([P, D], F32R, name=f"v{i}", tag=f"v{i}")
                    nc.vector.tensor_copy(t[:], ps[:])
                    vt.append(t)

                for n in range(NQ):
                    # Q^T chunk: [d, q] = WQK^T-lhsT @ X^T
                    qt = qpool.tile([P, KO, QC], F32R, tag="qt")
                    for m in range(KO):
                        ps = psA.tile([P, QC], F32, tag="mm")
                        for k in range(KO):
                            nc.tensor.matmul(
                                ps[:],
                                wqk[:, k, P * m:P * (m + 1)],
                                cur[n][:, k, :],
                                start=(k == 0),
                                stop=(k == KO - 1),
                            )
                        nc.vector.tensor_copy(qt[:, m, :], ps[:])

                    psy = [psY.tile([P, QC], F32, name=f"y{m}", tag=f"y{m}") for m in range(KO)]
                    pssum = psR.tile([1, QC], F32, tag="sum")
                    sl = strips[n]
                    for si, i in enumerate(sl):
                        pss = psS.tile([P, QC], F32, tag="s")
                        for k in range(KO):
                            nc.tensor.matmul(
                                pss[:],
                                cur[i // 4][:, k, P * (i % 4):P * (i % 4 + 1)],
                                qt[:, k, :],
                                start=(k == 0),
                                stop=(k == KO - 1),
                            )
                        for jj in range(4):
                            st = blocks[(i, 4 * n + jj)]
                            seg = pss[:, P * jj:P * (jj + 1)]
                            if st == "zero":
                                nc.vector.tensor_scalar_add(seg, seg, NEG)
                            elif st != "plain":
                                nc.vector.tensor_add(seg, seg, cm[:, st[1], :])
                        at = apool.tile([P, QC], F32R, tag="at")
                        nc.scalar.activation(
                            at[:], pss[:], mybir.ActivationFunctionType.Exp,
                            scale=16.0,
                        )
                        first = si == 0
                        last = si == len(sl) - 1
                        nc.tensor.matmul(
                            pssum[:], ones_r[:], at[:], start=first, stop=last
                        )
                        for m in range(KO):
                            nc.tensor.matmul(
                                psy[m][:],
                                vt[i][:, P * m:P * (m + 1)],
                                at[:],
                                start=first,
                                stop=last,
                            )
                    # normalize columns by 1/sum and add residual
                    r1 = npool.tile([1, QC], F32, tag="r1")
                    nc.vector.reciprocal(r1[:], pssum[:1, :])
                    rb = npool.tile([P, QC], F32, tag="rb")
                    nc.gpsimd.partition_broadcast(rb[:], r1[:1, :])
                    for m in range(KO):
                        t1 = npool.tile([P, QC], F32, tag="t1")
                        nc.vector.tensor_mul(t1[:], psy[m][:], rb[:])
                        nc.vector.tensor_add(
                            nxt[n][:, m, :],
                            cur[n][:, m, :].bitcast(F32),
                            t1[:],
                        )

            layer(xa, xb, w["wqk1"], w["wov1"])

            # layer 2 interleaved with logits per query chunk
            def layer2_and_logits():
                cur, nxt = xb, xa
                wqk, wov = w["wqk2"], w["wov2"]
                layer(cur, nxt, wqk, wov)
                for n in range(NQ):
                    for ii in range(4):
                        i = 4 * n + ii
                        for vci in range(NVC):
                            ps = psA.tile([P, VC], F32, tag="mm")
                            for k in range(KO):
                                nc.tensor.matmul(
                                    ps[:],
                                    nxt[n][:, k, P * ii:P * (ii + 1)],
                                    ut[vci][:, k, :],
                                    start=(k == 0),
                                    stop=(k == KO - 1),
                                )
                            ot = opool.tile([P, VC], F32, tag="ot")
                            if vci % 2 == 0:
                                nc.scalar.copy(ot[:], ps[:])
                            else:
                                nc.vector.tensor_copy(ot[:], ps[:])
                            nc.sync.dma_start(
                                out_d[P * i:P * (i + 1), VC * vci:VC * (vci + 1)],
                                ot[:],
                            )

            layer2_and_logits()

    nc.compile()
    return nc


def _structure_key(structure):
    blk = tuple(sorted((k, v) for k, v in structure["blocks"].items()))
    return (structure["strips"], blk, structure["n_mix"])


def _prepare(input, mask, E, P_pos, WQK1, WOV1, WQK2, WOV2, WF, U):
    tok = np.asarray(input).astype(np.int32)
    E = np.ascontiguousarray(np.asarray(E, dtype=np.float32))
    P_np = np.ascontiguousarray(np.asarray(P_pos, dtype=np.float32))
    structure, cm_tiles = _classify(np.asarray(mask))

    key = _structure_key(structure)
    if key not in _CACHE:
        _CACHE[key] = _build(structure)
    nc = _CACHE[key]

    wT = {
        "wqk1": np.ascontiguousarray(np.asarray(WQK1, np.float32).T),
        "wov1": np.ascontiguousarray(np.asarray(WOV1, np.float32).T),
        "wqk2": np.ascontiguousarray(np.asarray(WQK2, np.float32).T),
        "wov2": np.ascontiguousarray(np.asarray(WOV2, np.float32).T),
    }
    # fold FFN residual into the unembedding: logits = X2 @ (U + U WF)^T
    WF64 = np.asarray(WF, np.float64)
    U64 = np.asarray(U, np.float64)
    U2T = (U64 + U64 @ WF64).T.astype(np.float32)  # [D, V]

    in_maps = []
    for c in range(NCORES):
        b, sh = c // CPG, c % CPG
        in_maps.append(
            {
                "tok": tok[b],
                "emb": E,
                "pos": P_np,
                **wT,
                "ut": np.ascontiguousarray(U2T[:, sh * VSH:(sh + 1) * VSH]),
                "cmadd": cm_tiles[b],
            }
        )
    return nc, in_maps


def _assemble(results):
    logits = np.empty((B, S, V), dtype=np.float32)
    for c in range(NCORES):
        b, sh = c // CPG, c % CPG
        logits[b, :, sh * VSH:(sh + 1) * VSH] = results[c]["out"]
    return logits


def kernel(**inputs):
    nc, in_maps = _prepare(
        inputs["input"], inputs["mask"], inputs["E"], inputs["P"],
        inputs["WQK1"], inputs["WOV1"], inputs["WQK2"], inputs["WOV2"],
        inputs["WF"], inputs["U"],
    )
    res = run_bass_kernel_spmd(nc, in_maps, list(range(NCORES)))
    return _assemble(res.results)


# revision 10
# speedup vs baseline: 196.9677x; 196.9677x over previous
"""Trainium2 Bass kernel for nn_DualModel (B=2,S=2048,V=32000,D=256).

Sharding: 8 cores = 2 batch groups x 4 vocab shards. Core c handles
batch c//4 and vocab columns [8000*(c%4), 8000*(c%4+1)). Each core
runs the full transformer for its batch (embed + 2 attention layers)
and the logits GEMM for its vocab shard. No inter-core communication.

Device layout: activations kept transposed ("X^T" = [D, S]) in SBUF so
every matmul consumes them directly (lhsT for scores/values/logits, rhs
for Q projection). Attention probabilities are computed transposed
(A^T[t, q] = exp(16 * K_t . Q_q)), softmax normalization is folded in
after the A^T @ V matmul (it is linear per query column). The FFN
residual (X + X @ WF^T) is folded into the unembedding on the host:
logits = X2 @ (U + U @ WF)^T exactly.

Matmuls run in float32r (full PE rate); operands are rounded to f32r by
their producers (DMA from f32r DRAM, DVE/ACT writes to f32r tiles).
"""

import numpy as np

import concourse.bacc as bacc
import concourse.bass as bass
import concourse.mybir as mybir
import concourse.tile as tile
from concourse.bass_utils import run_bass_kernel_spmd
from concourse.masks import make_identity

P = 128
B, S, V, D = 2, 2048, 32000, 256
NCORES = 8
CPG = 4               # cores per batch group (vocab shards)
VSH = V // CPG        # 8000 vocab columns per core
KO = D // P           # 2 contraction chunks of 128
TB = S // P           # 16 token tiles / key chunks
QC = 512              # query-chunk width
NQ = S // QC          # 4 query chunks
VC = 500              # logits vocab chunk (PSUM bank limit 512 fp32)
NVC = VSH // VC       # 16
NEG = -1e30

F32 = mybir.dt.float32
F32R = mybir.dt.float32r
I32 = mybir.dt.int32

_CACHE = {}


def _classify(mask):
    """Per-128x128 block classification of mask[b][query, key], merged
    across batches into one SPMD-shared structure.

    Returns (structure, per_batch_tiles):
      structure: dict with
        strips[n]  -> ordered list of key-chunks i computed for q-chunk n
        blocks[(i, j)] -> 'plain' | 'zero' | ('add', tile_idx)
        n_mix      -> number of additive tiles
      per_batch_tiles: [B] list of np arrays [n_mix, P, P] fp32
    """
    stat = np.empty((B, TB, TB), dtype=np.int8)  # [b, key i, query j]
    mix = {}
    for b in range(B):
        mb = np.asarray(mask[b], dtype=bool)
        for j in range(TB):
            for i in range(TB):
                blk = mb[j * P:(j + 1) * P, i * P:(i + 1) * P]
                if blk.all():
                    stat[b, i, j] = 2
                elif not blk.any():
                    stat[b, i, j] = 0
                else:
                    stat[b, i, j] = 1
                    mix[(b, i, j)] = np.where(blk.T, 0.0, NEG).astype(np.float32)

    def tile_for(b, i, j):
        st = stat[b, i, j]
        if st == 2:
            return np.zeros((P, P), np.float32)
        if st == 0:
            return np.full((P, P), NEG, np.float32)
        return mix[(b, i, j)]

    blocks = {}
    dedupe = {}
    per_batch = [[] for _ in range(B)]
    for i in range(TB):
        for j in range(TB):
            sts = stat[:, i, j]
            if (sts == 2).all():
                blocks[(i, j)] = "plain"
            elif (sts == 0).all():
                blocks[(i, j)] = "zero"
            else:
                ts = [tile_for(b, i, j) for b in range(B)]
                key = tuple(t.tobytes() for t in ts)
                if key not in dedupe:
                    dedupe[key] = len(dedupe)
                    for b in range(B):
                        per_batch[b].append(ts[b])
                blocks[(i, j)] = ("add", dedupe[key])

    strips = []
    for n in range(NQ):
        js = range(4 * n, 4 * n + 4)
        strips.append(
            [i for i in range(TB) if any(blocks[(i, j)] != "zero" for j in js)]
        )
    n_mix = len(dedupe)
    structure = {
        "strips": tuple(tuple(s) for s in strips),
        "blocks": blocks,
        "n_mix": n_mix,
    }
    tiles = [
        np.stack(per_batch[b]) if n_mix else np.zeros((1, P, P), np.float32)
        for b in range(B)
    ]
    return structure, tiles


def _build(structure):
    strips = structure["strips"]
    blocks = structure["blocks"]
    n_mix = max(structure["n_mix"], 1)

    nc = bacc.Bacc("TRN2", target_bir_lowering=False, debug=False,
                   num_devices=NCORES)

    tok_d = nc.dram_tensor("tok", [S], I32, kind="ExternalInput")
    emb_d = nc.dram_tensor("emb", [V, D], F32, kind="ExternalInput")
    pos_d = nc.dram_tensor("pos", [S, D], F32, kind="ExternalInput")
    w_d = {
        n: nc.dram_tensor(n, [D, D], F32R, kind="ExternalInput")
        for n in ("wqk1", "wov1", "wqk2", "wov2")
    }
    ut_d = nc.dram_tensor("ut", [D, VSH], F32R, kind="ExternalInput")
    cm_d = nc.dram_tensor("cmadd", [n_mix, P, P], F32, kind="ExternalInput")
    out_d = nc.dram_tensor("out", [S, VSH], F32, kind="ExternalOutput")

    with tile.TileContext(nc) as tc:
        with (
            tc.tile_pool(name="cpool", bufs=1) as cpool,
            tc.tile_pool(name="xpool", bufs=1) as xpool,
            tc.tile_pool(name="upool", bufs=1) as upool,
            tc.tile_pool(name="qpool", bufs=2) as qpool,
            tc.tile_pool(name="epool", bufs=3) as epool,
            tc.tile_pool(name="apool", bufs=4) as apool,
            tc.tile_pool(name="npool", bufs=2) as npool,
            tc.tile_pool(name="opool", bufs=6) as opool,
            tc.tile_pool(name="psA", bufs=3, space="PSUM") as psA,
            tc.tile_pool(name="psL", bufs=2, space="PSUM") as psL,
            tc.tile_pool(name="psY", bufs=1, space="PSUM") as psY,
            tc.tile_pool(name="psR", bufs=1, space="PSUM") as psR,
        ):
            # ---- constants ----
            ident = cpool.tile([P, P], F32)
            make_identity(nc, ident[:])
            ones_f = cpool.tile([P, 1], F32)
            nc.vector.memset(ones_f[:], 1.0)
            ones_r = cpool.tile([P, 1], F32R)
            nc.vector.tensor_copy(ones_r[:], ones_f[:])
            toks = cpool.tile([P, TB], I32)
            nc.sync.dma_start(toks[:], tok_d.rearrange("(o p) -> p o", p=P))
            w = {}
            for nme in w_d:
                w[nme] = cpool.tile([P, KO, D], F32R, name=f"w_{nme}")
                nc.sync.dma_start(
                    w[nme][:], w_d[nme].rearrange("(ko p) n -> p ko n", p=P)
                )
            cm = cpool.tile([P, n_mix, P], F32)
            nc.sync.dma_start(cm[:], cm_d.rearrange("n p q -> p n q"))

            # activation buffers, [P, KO, QC] per query chunk
            xa = [xpool.tile([P, KO, QC], F32R, name=f"xa{n}", tag=f"xa{n}") for n in range(NQ)]
            xb = [xpool.tile([P, KO, QC], F32R, name=f"xb{n}", tag=f"xb{n}") for n in range(NQ)]

            # ---- embedding: X0^T = (E[tok] + pos)^T ----
            for i in range(TB):
                g = epool.tile([P, D], F32, tag="g")
                nc.gpsimd.indirect_dma_start(
                    out=g[:],
                    out_offset=None,
                    in_=emb_d[:],
                    in_offset=bass.IndirectOffsetOnAxis(ap=toks[:, i:i + 1], axis=0),
                )
                pt = epool.tile([P, D], F32, tag="pt")
                nc.sync.dma_start(pt[:], pos_d[P * i:P * (i + 1), :])
                x0 = epool.tile([P, D], F32, tag="x0")
                nc.vector.tensor_add(x0[:], g[:], pt[:])
                for k in range(KO):
                    tps = psL.tile([P, P], F32, tag="lg", name="tps")
                    nc.tensor.transpose(tps[:], x0[:, P * k:P * (k + 1)], ident[:])
                    nc.vector.tensor_copy(
                        xa[i // 4][:, k, P * (i % 4):P * (i % 4 + 1)], tps[:]
                    )

            # ---- attention layer, emitted per query chunk ----
            def make_vt(cur, vts, wov, i):
                ps = psA.tile([P, D], F32, tag="att", name="psv")
                for k in range(KO):
                    nc.tensor.matmul(
                        ps[:],
                        cur[i // 4][:, k, P * (i % 4):P * (i % 4 + 1)],
                        wov[:, k, :],
                        start=(k == 0),
                        stop=(k == KO - 1),
                    )
                nc.vector.tensor_copy(vts[i][:], ps[:])

            def layer_chunk(cur, nxt_tile, vts, vt_done, wqk, wov, n):
                sl = strips[n]
                for i in sl:
                    if i not in vt_done:
                        make_vt(cur, vts, wov, i)
                        vt_done.add(i)
                qt = qpool.tile([P, KO, QC], F32R, tag="qt", name="qt")
                for m in range(KO):
                    ps = psA.tile([P, QC], F32, tag="att", name="psq")
                    for k in range(KO):
                        nc.tensor.matmul(
                            ps[:],
                            wqk[:, k, P * m:P * (m + 1)],
                            cur[n][:, k, :],
                            start=(k == 0),
                            stop=(k == KO - 1),
                        )
                    nc.vector.tensor_copy(qt[:, m, :], ps[:])

                psy = [psY.tile([P, QC], F32, name=f"y{m}", tag=f"y{m}") for m in range(KO)]
                pssum = psR.tile([1, QC], F32, tag="sum", name="pssum")
                for si, i in enumerate(sl):
                    pss = psA.tile([P, QC], F32, tag="att", name="pss")
                    for k in range(KO):
                        nc.tensor.matmul(
                            pss[:],
                            cur[i // 4][:, k, P * (i % 4):P * (i % 4 + 1)],
                            qt[:, k, :],
                            start=(k == 0),
                            stop=(k == KO - 1),
                        )
                    for jj in range(4):
                        st = blocks[(i, 4 * n + jj)]
                        seg = pss[:, P * jj:P * (jj + 1)]
                        if st == "zero":
                            nc.vector.tensor_scalar_add(seg, seg, NEG)
                        elif st != "plain":
                            nc.vector.tensor_add(seg, seg, cm[:, st[1], :])
                    at = apool.tile([P, QC], F32R, tag="at", name="at")
                    nc.scalar.activation(
                        at[:], pss[:], mybir.ActivationFunctionType.Exp,
                        scale=16.0,
                    )
                    first = si == 0
                    last = si == len(sl) - 1
                    nc.tensor.matmul(
                        pssum[:], ones_r[:], at[:], start=first, stop=last
                    )
                    for m in range(KO):
                        nc.tensor.matmul(
                            psy[m][:],
                            vts[i][:, P * m:P * (m + 1)],
                            at[:],
                            start=first,
                            stop=last,
                        )
                # normalize columns by 1/sum and add residual
                r1 = npool.tile([1, QC], F32, tag="r1", name="r1")
                nc.vector.reciprocal(r1[:], pssum[:1, :])
                rb = npool.tile([P, QC], F32, tag="rb", name="rb")
                nc.gpsimd.partition_broadcast(rb[:], r1[:1, :])
                for m in range(KO):
                    t1 = npool.tile([P, QC], F32, tag="t1", name="t1")
                    nc.vector.tensor_mul(t1[:], psy[m][:], rb[:])
                    nc.vector.tensor_add(
                        nxt_tile[:, m, :],
                        cur[n][:, m, :].bitcast(F32),
                        t1[:],
                    )

            def logits_chunk(x2, ut, n):
                for ii in range(4):
                    i = 4 * n + ii
                    for vci in range(NVC):
                        ps = psL.tile([P, VC], F32, tag="lg", name="psl")
                        for k in range(KO):
                            nc.tensor.matmul(
                                ps[:],
                                x2[:, k, P * ii:P * (ii + 1)],
                                ut[vci][:, k, :],
                                start=(k == 0),
                                stop=(k == KO - 1),
                            )
                        ot = opool.tile([P, VC], F32, tag="ot", name="ot")
                        if vci % 3 == 2:
                            nc.vector.tensor_copy(ot[:], ps[:])
                        else:
                            nc.scalar.copy(ot[:], ps[:])
                        nc.sync.dma_start(
                            out_d[P * i:P * (i + 1), VC * vci:VC * (vci + 1)],
                            ot[:],
                        )

            vt1 = [xpool.tile([P, D], F32R, name=f"v1_{i}", tag=f"v1_{i}") for i in range(TB)]
            vt2 = [xpool.tile([P, D], F32R, name=f"v2_{i}", tag=f"v2_{i}") for i in range(TB)]
            vt1_done, vt2_done = set(), set()

            # software pipeline: L1(n) -> L2(n) -> logits(n)
            ut = None
            for n in range(NQ):
                layer_chunk(xa, xb[n], vt1, vt1_done, w["wqk1"], w["wov1"], n)
                if n == 0:
                    # unembedding loads: issued here so embed/L1 DMAs go first;
                    # they fill DMA idle time during the attention phase
                    ut = []
                    for vci in range(NVC):
                        t = upool.tile([P, KO, VC], F32R, name=f"ut{vci}", tag=f"ut{vci}")
                        nc.sync.dma_start(
                            t[:],
                            ut_d.rearrange("(ko p) v -> p ko v", p=P)[
                                :, :, vci * VC:(vci + 1) * VC
                            ],
                        )
                        ut.append(t)
                x2 = qpool.tile([P, KO, QC], F32R, tag="x2", name="x2")
                layer_chunk(xb, x2, vt2, vt2_done, w["wqk2"], w["wov2"], n)
                logits_chunk(x2, ut, n)

    nc.compile()
    return nc


def _structure_key(structure):
    blk = tuple(sorted((k, v) for k, v in structure["blocks"].items()))
    return (structure["strips"], blk, structure["n_mix"])


def _prepare(input, mask, E, P_pos, WQK1, WOV1, WQK2, WOV2, WF, U):
    tok = np.asarray(input).astype(np.int32)
    E = np.ascontiguousarray(np.asarray(E, dtype=np.float32))
    P_np = np.ascontiguousarray(np.asarray(P_pos, dtype=np.float32))
    structure, cm_tiles = _classify(np.asarray(mask))

    key = _structure_key(structure)
    if key not in _CACHE:
        _CACHE[key] = _build(structure)
    nc = _CACHE[key]

    wT = {
        "wqk1": np.ascontiguousarray(np.asarray(WQK1, np.float32).T),
        "wov1": np.ascontiguousarray(np.asarray(WOV1, np.float32).T),
        "wqk2": np.ascontiguousarray(np.asarray(WQK2, np.float32).T),
        "wov2": np.ascontiguousarray(np.asarray(WOV2, np.float32).T),
    }
    # fold FFN residual into the unembedding: logits = X2 @ (U + U WF)^T
    WF64 = np.asarray(WF, np.float64)
    U64 = np.asarray(U, np.float64)
    U2T = (U64 + U64 @ WF64).T.astype(np.float32)  # [D, V]

    in_maps = []
    for c in range(NCORES):
        b, sh = c // CPG, c % CPG
        in_maps.append(
            {
                "tok": tok[b],
                "emb": E,
                "pos": P_np,
                **wT,
                "ut": np.ascontiguousarray(U2T[:, sh * VSH:(sh + 1) * VSH]),
                "cmadd": cm_tiles[b],
            }
        )
    return nc, in_maps


def _assemble(results):
    logits = np.empty((B, S, V), dtype=np.float32)
    for c in range(NCORES):
        b, sh = c // CPG, c % CPG
        logits[b, :, sh * VSH:(sh + 1) * VSH] = results[c]["out"]
    return logits


def kernel(**inputs):
    nc, in_maps = _prepare(
        inputs["input"], inputs["mask"], inputs["E"], inputs["P"],
        inputs["WQK1"], inputs["WOV1"], inputs["WQK2"], inputs["WOV2"],
        inputs["WF"], inputs["U"],
    )
    res = run_bass_kernel_spmd(nc, in_maps, list(range(NCORES)))
    return _assemble(res.results)
